# revision 1
# baseline (speedup 1.0000x reference)
"""Trainium2 Bass kernel for nn_MultiHeadAttention_86079734546451.

Sharding: data-parallel over batch B=16 across 8 cores (2 batches/core).
All weights replicated. No collectives.

Per-core math (B_loc=2, D=512, N=128 nodes, S=14, L=12, H=8, dh=64):
  qt = q.transpose -> [S,N,D] tokens; qh = qt@Wq.T*scale+bq (folded into W on host)
  scoresT[m,n] = kh[m]. qh[n] per (s,h)   (m,n node indices)
  softmax over n (the query axis!) == free-axis softmax in the [m,n] layout
  bias = einsum('lnmh,sl->snmh', ab, Wd) done as one K=96 matmul per chunk
     with a host-built block-diagonal kron weight, then bf16 xbar-transposed
     into a [m, (n,s)] layout for the DVE add. (bd cancels in softmax.)
  out^T[d,n] = (vh/Z) @ E  per (s,h); output projection from the d-major y^T.
"""

import sys

sys.path.insert(0, "/opt/trn_rl_repo")

from contextlib import ExitStack

import numpy as np

import concourse.bass as bass
import concourse.mybir as mybir
import concourse.tile as tile
from concourse import bacc

f32 = mybir.dt.float32
f32r = mybir.dt.float32r
bf16 = mybir.dt.bfloat16
AF = mybir.ActivationFunctionType

# Problem constants
B_LOC = 2          # batches per core
D = 512
N = 128            # nodes
S = 14             # seq (out of Dimension1 proj)
SP = 16            # padded S for xbar transpose
L = 12
H = 8
DH = 64            # head dim
TOK = N * S        # 1792 tokens per batch, (n, s) order
C = 4              # 128-chunks of D
NCORES = 8

# bias-projection chunking: ch = n-block of 8
NB = 8             # n per chunk
NCH = N // NB      # 16 chunks per batch


def emit_kernel(ctx: ExitStack, tc: "tile.TileContext", io: dict):
    nc = tc.nc

    q_d, k_d, v_d, ab_d = io["q"], io["k"], io["v"], io["ab"]
    out_d = io["out"]

    # ---------------- pools ----------------
    wpool = ctx.enter_context(tc.tile_pool(name="wpool", bufs=1))
    xin = ctx.enter_context(tc.tile_pool(name="xin", bufs=4))
    qkh = ctx.enter_context(tc.tile_pool(name="qkh", bufs=8))
    vhp = ctx.enter_context(tc.tile_pool(name="vhp", bufs=1))
    ytp = ctx.enter_context(tc.tile_pool(name="ytp", bufs=4))
    abp = ctx.enter_context(tc.tile_pool(name="abp", bufs=2))
    bsbp = ctx.enter_context(tc.tile_pool(name="bsbp", bufs=1))
    tmpp = ctx.enter_context(tc.tile_pool(name="tmpp", bufs=4))
    drp = ctx.enter_context(tc.tile_pool(name="drp", bufs=4, space="DRAM"))
    tbp = ctx.enter_context(tc.tile_pool(name="tbp", bufs=1))
    ebp = ctx.enter_context(tc.tile_pool(name="ebp", bufs=2))
    zrp = ctx.enter_context(tc.tile_pool(name="zrp", bufs=4))
    vpp = ctx.enter_context(tc.tile_pool(name="vpp", bufs=4))
    osbp = ctx.enter_context(tc.tile_pool(name="osbp", bufs=2))

    pp = ctx.enter_context(tc.tile_pool(name="pp", bufs=2, space="PSUM"))
    pb = ctx.enter_context(tc.tile_pool(name="pb", bufs=2, space="PSUM"))
    scp = ctx.enter_context(tc.tile_pool(name="scp", bufs=2, space="PSUM"))
    avp = ctx.enter_context(tc.tile_pool(name="avp", bufs=2, space="PSUM"))

    # ---------------- weights (once) ----------------
    wq = []
    wk = []
    wv = []
    wo = []
    for c in range(C):
        wq_c = wpool.tile([128, D], f32r, name=f"wq{c}", tag=f"wq{c}")
        nc.scalar.dma_start(wq_c[:], io["wqT"][c * 128:(c + 1) * 128, :].bitcast(f32r))
        wq.append(wq_c)
        wk_c = wpool.tile([128, D], f32r, name=f"wk{c}", tag=f"wk{c}")
        nc.scalar.dma_start(wk_c[:], io["wkT"][c * 128:(c + 1) * 128, :].bitcast(f32r))
        wk.append(wk_c)
        wv_c = wpool.tile([128, D], f32r, name=f"wv{c}", tag=f"wv{c}")
        nc.scalar.dma_start(wv_c[:], io["wvT"][c * 128:(c + 1) * 128, :].bitcast(f32r))
        wv.append(wv_c)
        # Wo in bf16 (O-proj runs bf16 against bf16 yT); gpsimd DMA casts
        wo_c = wpool.tile([128, D], bf16, name=f"wo{c}", tag=f"wo{c}")
        nc.gpsimd.dma_start(wo_c[:], io["woT"][c * 128:(c + 1) * 128, :])
        wo.append(wo_c)

    wdk = wpool.tile([48, 128], f32r, name="wdk", tag="wdk")
    nc.scalar.dma_start(wdk[:], io["wdk"][:].bitcast(f32r))

    # per-dout-chunk bias columns for Q/K (ACT bias operand, [128,1])
    bq_t = wpool.tile([128, C], f32, name="bq_t", tag="bq_t")
    bk_t = wpool.tile([128, C], f32, name="bk_t", tag="bk_t")
    for c in range(C):
        nc.scalar.dma_start(bq_t[:, c:c + 1],
                          io["bq"][c * 128:(c + 1) * 128].unsqueeze(1))
        nc.scalar.dma_start(bk_t[:, c:c + 1],
                          io["bk"][c * 128:(c + 1) * 128].unsqueeze(1))

    # broadcast bv/bo along partitions via K=1 ones-matmul
    ones = wpool.tile([1, 128], f32, name="ones", tag="ones")
    nc.vector.memset(ones[:], 1.0)
    bv_st = wpool.tile([1, D], f32, name="bv_st", tag="bv_st")
    nc.scalar.dma_start(bv_st[:], io["bv"].unsqueeze(0))
    bo_st = wpool.tile([1, D], f32, name="bo_st", tag="bo_st")
    nc.scalar.dma_start(bo_st[:], io["bo"].unsqueeze(0))
    bvb = wpool.tile([128, D], f32, name="bvb", tag="bvb")
    bob = wpool.tile([128, D], f32, name="bob", tag="bob")
    for src, dst in ((bv_st, bvb), (bo_st, bob)):
        pst = pp.tile([128, D], f32, tag="pp", name="p_bcast")
        nc.tensor.matmul(pst[:], lhsT=ones[:], rhs=src[:1, :] if src.shape[0] > 1 else src[:], start=True, stop=True)
        nc.vector.tensor_copy(dst[:], pst[:])

    # ---------------- per-batch body ----------------
    for b in range(B_LOC):
        # ---- P1: Q and K projections -> d-major qhT/khT [dout, tok] bf16
        qh = []
        kh = []
        for (src_d, wts, bias_t, dst_list) in (
            (q_d, wq, bq_t, qh),
            (k_d, wk, bk_t, kh),
        ):
            xq = []
            for ci in range(C):
                x_c = xin.tile([128, TOK], f32r, tag="xin", name=f"x{ci}")
                nc.scalar.dma_start(
                    x_c[:],
                    src_d[b, ci * 128:(ci + 1) * 128].rearrange("p n s -> p (n s)").bitcast(f32r))
                xq.append(x_c)
            for co in range(C):
                h_c = qkh.tile([128, TOK], bf16, tag="qkh", name=f"h{co}")
                for tb in range(4):
                    ps = pp.tile([128, 448], f32, tag="pp", name="ps_qk")
                    for ci in range(C):
                        nc.tensor.matmul(
                            ps[:],
                            lhsT=wts[ci][:, co * 128:(co + 1) * 128],
                            rhs=xq[ci][:, tb * 448:(tb + 1) * 448],
                            start=(ci == 0), stop=(ci == C - 1))
                    nc.scalar.activation(h_c[:, tb * 448:(tb + 1) * 448], ps[:],
                                         AF.Identity, bias=bias_t[:, co:co + 1],
                                         scale=1.0)
                dst_list.append(h_c)

        # ---- P1b: V projection -> token-major vh [n, (s,hd)] bf16 (+bv)
        xv = []
        for ci in range(C):
            xv_c = xin.tile([128, TOK], f32r, tag="xin", name=f"xv{ci}")
            nc.scalar.dma_start(
                xv_c[:],
                v_d[b, ci * 128:(ci + 1) * 128].rearrange("p n s -> p (n s)").bitcast(f32r))
            xv.append(xv_c)
        vh = vhp.tile([128, S * D], bf16, name="vh")
        for s in range(S):
            ps = pp.tile([128, D], f32, tag="pp", name="ps_v")
            for ci in range(C):
                nc.tensor.matmul(
                    ps[:],
                    lhsT=xv[ci][:, s::S],
                    rhs=wv[ci][:],
                    start=(ci == 0), stop=(ci == C - 1))
            nc.vector.tensor_add(vh[:, s * D:(s + 1) * D], ps[:], bvb[:])

        # ---- P2: bias projection + transpose into tb_t
        # tb_t cols: (ch:16)(h:8)(nb:8)(s:16)   [bf16]
        # psum/bsb rows: (j:4)(s:32-pad); m = j*32 + mi
        tb_t = tbp.tile([128, NCH * 1024], bf16, name="tb_t")
        for ch in range(NCH):  # n-blocks of 8
            abt = abp.tile([48, NB * 256], f32r, tag="abt", name="abt")
            for l in range(L):
                nc.scalar.dma_start(
                    abt[l * 4:(l + 1) * 4, :],
                    ab_d[b, l, ch * NB:(ch + 1) * NB].rearrange(
                        "n (j mi) h -> j n (mi h)", j=4).bitcast(f32r))
            # bsb cols: (h:8)(nb:8)(mi:32 + 96 pad to one xbar tile)
            bsb_t = bsbp.tile([128, 8192], bf16, tag="bsb", name="bsb")
            for quar in range(4):
                ps = pb.tile([128, 512], f32, tag="pb", name="ps_b")
                nc.tensor.matmul(
                    ps[:],
                    lhsT=wdk[:],
                    rhs=abt[:, quar * 512:(quar + 1) * 512],
                    start=True, stop=True)
                # psum cols (nb2:2)(mi:32)(h:8) -> bsb cols (h)(nb2)(mi)
                nc.scalar.activation(_bsb_out_ap(bsb_t, quar * 2),
                                     _psum_perm_ap(ps), AF.Copy)
            if b == 0 and ch == 0 and "dbg_bsb" in io:
                nc.scalar.dma_start(io["dbg_bsb"], bsb_t[:])
            for j in range(4):
                # xbar: out base must be 0 -> bounce via tmp, then shift
                # partitions with a plain sbuf->sbuf DMA (2KB runs)
                tmp_t = tmpp.tile([32, 1024], bf16, tag="tmp", name="tmp_t")
                nc.sync.dma_start(
                    tmp_t.rearrange("p (hn s) -> p hn s", s=SP),
                    bsb_t[j * 32:j * 32 + SP, :],
                    transpose=True)
                if b == 0 and ch == 0 and j == 0 and "dbg_tmp" in io:
                    nc.scalar.dma_start(io["dbg_tmp"], tmp_t[:])
                # bounce via DRAM: concurrent xbar-transpose + SBUF->SBUF DMA
                # corrupts data (HW hazard), DRAM-routed DMAs are safe
                dsc = drp.tile([32, 1024], bf16, tag="dsc", name="dsc")
                nc.scalar.dma_start(dsc[:], tmp_t[:])
                nc.scalar.dma_start(
                    tb_t[j * 32:(j + 1) * 32, ch * 1024:(ch + 1) * 1024],
                    dsc[:])

        if b == 0 and "dbg_qh" in io:
            for cc in range(C):
                nc.scalar.dma_start(io["dbg_qh"][cc], qh[cc][:])
                nc.scalar.dma_start(io["dbg_kh"][cc], kh[cc][:])
            nc.scalar.dma_start(io["dbg_vh"], vh[:])
            nc.scalar.dma_start(io["dbg_tb"], tb_t[:])

        # ---- P3: attention per (h, s)
        yt = [ytp.tile([128, TOK], bf16, tag="ytp", name=f"yt{c}")
              for c in range(C)]
        for h in range(H):
            c = h // 2
            hb = (h % 2) * DH
            ebt = ebp.tile([128, S * 128], bf16, name="ebt")
            zt = zrp.tile([128, SP], f32, tag="z", name="zt")
            for s in range(S):
                sc_t = scp.tile([128, 128], f32, tag="sc", name="sc_t")
                nc.tensor.matmul(
                    sc_t[:],
                    lhsT=kh[c][hb:hb + DH, s::S],
                    rhs=qh[c][hb:hb + DH, s::S],
                    start=True, stop=True)
                # += bias: tb cols (chpair, h, nb2, s): n = chp*8+nb
                bias_ap = _tb_bias_ap(tb_t, h, s)
                nc.vector.tensor_add(
                    sc_t.rearrange("p (a b) -> p a b", a=NCH // 2),
                    sc_t.rearrange("p (a b) -> p a b", a=NCH // 2),
                    bias_ap)
                nc.scalar.activation(ebt[:, s * 128:(s + 1) * 128], sc_t[:],
                                     AF.Exp, accum_out=zt[:, s:s + 1])
            if b == 0 and h == 0 and "dbg_eb" in io:
                nc.scalar.dma_start(io["dbg_eb"], ebt[:])
                nc.scalar.dma_start(io["dbg_z"], zt[:])
            rt = zrp.tile([128, SP], f32, tag="r", name="rt")
            nc.vector.reciprocal(rt[:, :S], zt[:, :S])
            for s in range(S):
                vp = vpp.tile([128, DH], bf16, tag="vp", name="vp")
                nc.gpsimd.tensor_scalar_mul(
                    vp[:], vh[:, s * D + h * DH: s * D + (h + 1) * DH],
                    rt[:, s:s + 1])
                av_t = avp.tile([128, 128], f32, tag="av", name="av_t")
                nc.tensor.matmul(
                    av_t[hb:hb + DH, :],
                    lhsT=vp[:],
                    rhs=ebt[:, s * 128:(s + 1) * 128],
                    start=True, stop=True)
                nc.vector.tensor_copy(yt[c][hb:hb + DH, s * 128:(s + 1) * 128],
                                      av_t[hb:hb + DH, :])

        # ---- P4: output projection
        for s in range(S):
            ps = pp.tile([128, D], f32, tag="pp", name="ps_o")
            for ci in range(C):
                nc.tensor.matmul(
                    ps[:],
                    lhsT=yt[ci][:, s * 128:(s + 1) * 128],
                    rhs=wo[ci][:],
                    start=(ci == 0), stop=(ci == C - 1))
            osb = osbp.tile([128, D], f32, tag="osb", name="osb")
            nc.vector.tensor_add(osb[:], ps[:], bob[:])
            nc.scalar.dma_start(out_d[b, s], osb[:])


def _psum_perm_ap(ps):
    # psum [128, 512] with cols (nb2:2, mi:32, h:8) read in (h, nb2, mi) order
    return ps.rearrange("p (nb mi h) -> p h nb mi", nb=2, mi=32, h=8)


def _bsb_out_ap(bsb_t, nbp):
    # bsb cols (h:8, nb:8, mi:128[32 real + 96 pad]); write nb block [nbp, nbp+2)
    return bsb_t.rearrange("p (h nb mi) -> p h nb mi", h=8, nb=8, mi=128)[
        :, :, nbp:nbp + 2, :32]


def _tb_bias_ap(tb_t, h, s):
    # tb cols (ch:16, h:8, nb:8, s:16); bias[m, n] for n=(ch*8+nb)
    v = tb_t.rearrange("p (ch h nb s) -> p ch h nb s", ch=16, h=8, nb=8)
    return v[:, :, h, :, s]


def build_nc(debug_taps=False):
    nc = bacc.Bacc("TRN2", target_bir_lowering=False, debug=False,
                   num_devices=NCORES)
    io = {}
    io["q"] = nc.dram_tensor("q", [B_LOC, D, N, S], f32, kind="ExternalInput").ap()
    io["k"] = nc.dram_tensor("k", [B_LOC, D, N, S], f32, kind="ExternalInput").ap()
    io["v"] = nc.dram_tensor("v", [B_LOC, D, N, S], f32, kind="ExternalInput").ap()
    io["ab"] = nc.dram_tensor("ab", [B_LOC, L, N, N, H], f32,
                              kind="ExternalInput").ap()
    for nm in ("wqT", "wkT", "wvT", "woT"):
        io[nm] = nc.dram_tensor(nm, [D, D], f32, kind="ExternalInput").ap()
    for nm in ("bq", "bk", "bv", "bo"):
        io[nm] = nc.dram_tensor(nm, [D], f32, kind="ExternalInput").ap()
    io["wdk"] = nc.dram_tensor("wdk", [48, 128], f32, kind="ExternalInput").ap()
    io["out"] = nc.dram_tensor("out", [B_LOC, S, N, D], f32,
                               kind="ExternalOutput").ap()
    if debug_taps:
        io["dbg_qh"] = [nc.dram_tensor(f"dbg_qh{c}", [128, TOK], bf16,
                                       kind="ExternalOutput").ap()
                        for c in range(C)]
        io["dbg_kh"] = [nc.dram_tensor(f"dbg_kh{c}", [128, TOK], bf16,
                                       kind="ExternalOutput").ap()
                        for c in range(C)]
        io["dbg_vh"] = nc.dram_tensor("dbg_vh", [128, S * D], bf16,
                                      kind="ExternalOutput").ap()
        io["dbg_tb"] = nc.dram_tensor("dbg_tb", [128, NCH * 1024], bf16,
                                      kind="ExternalOutput").ap()
        io["dbg_eb"] = nc.dram_tensor("dbg_eb", [128, S * 128], bf16,
                                      kind="ExternalOutput").ap()
        io["dbg_z"] = nc.dram_tensor("dbg_z", [128, SP], f32,
                                     kind="ExternalOutput").ap()
        io["dbg_bsb"] = nc.dram_tensor("dbg_bsb", [128, 8192], bf16,
                                       kind="ExternalOutput").ap()
        io["dbg_tmp"] = nc.dram_tensor("dbg_tmp", [32, 1024], bf16,
                                       kind="ExternalOutput").ap()

    with tile.TileContext(nc) as tc:
        with ExitStack() as ctx:
            emit_kernel(ctx, tc, io)
    nc.compile()
    return nc


def host_prep(Wq, bq, Wk, bk, Wv, bv, Wd, bd, Wo, bo):
    """Pre-transpose weights; fold the qk scale into Wq; build the
    block-diagonal kron weight for the bias L->S projection. bd cancels
    in the softmax (constant along the normalized axis) and is dropped."""
    scale = (D // H) ** -0.5
    prep = {
        "wqT": np.ascontiguousarray((Wq * scale).T).astype(np.float32),
        "wkT": np.ascontiguousarray(Wk.T).astype(np.float32),
        "wvT": np.ascontiguousarray(Wv.T).astype(np.float32),
        "woT": np.ascontiguousarray(Wo.T).astype(np.float32),
        "bq": (bq * scale).astype(np.float32),
        "bk": bk.astype(np.float32),
        "bv": bv.astype(np.float32),
        "bo": bo.astype(np.float32),
    }
    wdk = np.zeros((48, 128), np.float32)
    for j in range(4):
        # rows l*4+j, cols j*32+s
        wdk[np.arange(L) * 4 + j, j * 32:j * 32 + S] = np.asarray(Wd).T
    prep["wdk"] = wdk
    return prep


_NC_CACHE = None


def run(q, k, v, attn_bias, Wq, bq, Wk, bk, Wv, bv, Wd, bd, Wo, bo,
        trace=False, **trace_kwargs):
    global _NC_CACHE
    from concourse.bass_utils import run_bass_kernel_spmd

    if _NC_CACHE is None:
        _NC_CACHE = build_nc()
    nc = _NC_CACHE

    prep = host_prep(Wq, bq, Wk, bk, Wv, bv, Wd, bd, Wo, bo)
    q = np.asarray(q, np.float32)
    k = np.asarray(k, np.float32)
    v = np.asarray(v, np.float32)
    ab = np.asarray(attn_bias, np.float32)

    in_maps = []
    for i in range(NCORES):
        sl = slice(i * B_LOC, (i + 1) * B_LOC)
        in_maps.append({
            "q": np.ascontiguousarray(q[sl]),
            "k": np.ascontiguousarray(k[sl]),
            "v": np.ascontiguousarray(v[sl]),
            "ab": np.ascontiguousarray(ab[sl]),
            **prep,
        })
    res = run_bass_kernel_spmd(nc, in_maps, list(range(NCORES)), trace=trace,
                               **trace_kwargs)
    out = np.concatenate([res.results[i]["out"] for i in range(NCORES)], axis=0)
    return out, res


def kernel(**inputs):
    return run(**inputs)[0]



# revision 7
# speedup vs baseline: 5.2141x; 5.2141x over previous
"""Trainium2 Bass kernel for nn_MultiHeadAttention_86079734546451.

Sharding: data-parallel over batch B=16 across 8 cores (2 batches/core).
All weights replicated. No collectives.

Per-core math (B_loc=2, D=512, N=128 nodes, S=14, L=12, H=8, dh=64):
  qh/kh = d-major [dout, (n,s)] bf16 projections (scale folded into Wq).
  bias  = einsum('lnmh,sl->mns', ab, Wd) via per-(h,nb) kron matmuls:
     lhsT = host-pretransposed ab block [(l,ni)=96, m=128] bf16,
     rhs  = kron Wd [96, (ni,s16)];  4 blocks -> one [128,512] psum tile,
     ACT-copied (strided, dropping the s-pad) into the score psum as the
     accumulation INIT; score matmuls then accumulate with start=False.
     (bd cancels in the softmax and is dropped.)
  softmax over the query axis == free-axis softmax in the [m,(s,n)] layout:
     one exp [128,1792] -> ebt bf16, 3D reduce -> Z, reciprocal,
     gpsimd broadcast-multiply onto vh -> vp; AV matmuls per (h,s);
     O-projection from the d-major yt tiles.
"""

import sys

sys.path.insert(0, "/opt/trn_rl_repo")

from contextlib import ExitStack

import numpy as np
import ml_dtypes

import concourse.bass as bass
import concourse.mybir as mybir
import concourse.tile as tile
from concourse import bacc
from concourse.bass import broadcast_tensor_aps

f32 = mybir.dt.float32
bf16 = mybir.dt.bfloat16
AF = mybir.ActivationFunctionType

# Problem constants
B_LOC = 2          # batches per core
D = 512
N = 128            # nodes
S = 14             # seq
SP = 16            # padded S in the kron weight
L = 12
H = 8
DH = 64            # head dim
TOK = N * S        # 1792 tokens per batch, (n, s) order
C = 4              # 128-chunks of D
NCORES = 8
NI = 8             # n per kron block
NBK = N // NI      # 16 kron blocks
K96 = L * NI       # kron contraction size


def emit_kernel(ctx: ExitStack, tc: "tile.TileContext", io: dict):
    nc = tc.nc

    q_d, k_d, v_d, ab_d = io["q"], io["k"], io["v"], io["abk"]
    out_d = io["out"]

    # ---------------- pools ----------------
    wpool = ctx.enter_context(tc.tile_pool(name="wpool", bufs=1))
    xin = ctx.enter_context(tc.tile_pool(name="xin", bufs=16))
    qkh = ctx.enter_context(tc.tile_pool(name="qkh", bufs=16))
    vhp = ctx.enter_context(tc.tile_pool(name="vhp", bufs=2))
    abp = ctx.enter_context(tc.tile_pool(name="abp", bufs=3))
    ebp = ctx.enter_context(tc.tile_pool(name="ebp", bufs=2))
    vpp = ctx.enter_context(tc.tile_pool(name="vpp", bufs=2))
    zrp = ctx.enter_context(tc.tile_pool(name="zrp", bufs=4))
    ytp = ctx.enter_context(tc.tile_pool(name="ytp", bufs=4))
    osbp = ctx.enter_context(tc.tile_pool(name="osbp", bufs=3))

    pp = ctx.enter_context(tc.tile_pool(name="pp", bufs=2, space="PSUM"))
    scp = ctx.enter_context(tc.tile_pool(name="scp", bufs=1, space="PSUM"))
    avp = ctx.enter_context(tc.tile_pool(name="avp", bufs=2, space="PSUM"))

    # ---------------- weights (once) ----------------
    wq, wk, wv, wo = [], [], [], []
    for c in range(C):
        for nm, lst in (("wqT", wq), ("wkT", wk), ("wvT", wv), ("woT", wo)):
            w_c = wpool.tile([128, D], bf16, name=f"{nm}{c}", tag=f"{nm}{c}")
            nc.scalar.dma_start(w_c[:], io[nm][c * 128:(c + 1) * 128, :])
            lst.append(w_c)

    wdh = wpool.tile([K96, NI * SP], bf16, name="wdh", tag="wdh")
    nc.scalar.dma_start(wdh[:], io["wdh"][:])

    # packed per-partition consts: cols 0-3 bq chunks, 4-7 bk chunks
    cst = wpool.tile([128, 8], f32, name="cst", tag="cst")
    nc.scalar.dma_start(cst[:], io["cst"][:])

    # broadcast bv/bo along partitions via K=1 ones-matmul
    ones = wpool.tile([1, 128], f32, name="ones", tag="ones")
    nc.vector.memset(ones[:], 1.0)
    bv_st = wpool.tile([1, D], f32, name="bv_st", tag="bv_st")
    nc.scalar.dma_start(bv_st[:], io["bv"].unsqueeze(0))
    bo_st = wpool.tile([1, D], f32, name="bo_st", tag="bo_st")
    nc.scalar.dma_start(bo_st[:], io["bo"].unsqueeze(0))
    bvb = wpool.tile([128, D], f32, name="bvb", tag="bvb")
    bob = wpool.tile([128, D], f32, name="bob", tag="bob")
    for src, dst in ((bv_st, bvb), (bo_st, bob)):
        pst = pp.tile([128, D], f32, tag="pp", name="p_bcast")
        nc.tensor.matmul(pst[:], lhsT=ones[:], rhs=src[:], start=True, stop=True)
        nc.vector.tensor_copy(dst[:], pst[:])

    def load_x(b):
        xs = {}
        for (src_d, nm) in ((q_d, "xq"), (k_d, "xk"), (v_d, "xv")):
            lst = []
            for ci in range(C):
                x_c = xin.tile([128, TOK], bf16, tag="xin", name=f"{nm}{ci}")
                nc.sync.dma_start(
                    x_c[:],
                    src_d[b, ci * 128:(ci + 1) * 128].rearrange(
                        "p n s -> p (n s)"))
                lst.append(x_c)
            xs[nm] = lst
        return xs

    def load_ab(b, h):
        ab_h = abp.tile([K96, NBK * 128], bf16, tag="abt", name=f"ab{h}")
        nc.sync.dma_start(ab_h[:], ab_d[b, h])
        return ab_h

    # ---------------- per-batch body ----------------
    xs = load_x(0)
    for b in range(B_LOC):
        xq, xk, xv = xs["xq"], xs["xk"], xs["xv"]

        # ---- P1: Q and K projections -> d-major [dout, (n,s)] bf16
        qh, kh = [], []
        for pi, (xin_l, wts, dst_list) in enumerate(
                ((xq, wq, qh), (xk, wk, kh))):
            for co in range(C):
                h_c = qkh.tile([128, TOK], bf16, tag="qkh", name=f"h{co}")
                for tb in range(4):
                    ps = pp.tile([128, 448], f32, tag="pp", name="ps_qk")
                    for ci in range(C):
                        nc.tensor.matmul(
                            ps[:],
                            lhsT=wts[ci][:, co * 128:(co + 1) * 128],
                            rhs=xin_l[ci][:, tb * 448:(tb + 1) * 448],
                            start=(ci == 0), stop=(ci == C - 1))
                    eng = nc.scalar if pi == 0 else nc.vector
                    if pi == 0:
                        nc.scalar.activation(
                            h_c[:, tb * 448:(tb + 1) * 448], ps[:],
                            AF.Identity, bias=cst[:, co:co + 1], scale=1.0)
                    else:
                        nc.vector.tensor_scalar_add(
                            h_c[:, tb * 448:(tb + 1) * 448], ps[:],
                            cst[:, 4 + co:4 + co + 1])
                dst_list.append(h_c)

        # ---- P1b: V projection -> token-major vh [n, (s,d)] bf16 (+bv)
        vh = vhp.tile([128, S * D], bf16, tag="vh", name="vh")
        for s in range(S):
            ps = pp.tile([128, D], f32, tag="pp", name="ps_v")
            for ci in range(C):
                nc.tensor.matmul(
                    ps[:],
                    lhsT=xv[ci][:, s::S],
                    rhs=wv[ci][:],
                    start=(ci == 0), stop=(ci == C - 1))
            nc.vector.tensor_add(vh[:, s * D:(s + 1) * D], ps[:], bvb[:])

        # prefetch next batch's inputs (sync queue, before this batch's outs)
        if b + 1 < B_LOC:
            xs = load_x(b + 1)

        # ---- P3: attention per head
        yt = [ytp.tile([128, TOK], bf16, tag="ytp", name=f"yt{c}")
              for c in range(C)]
        abL = {hh: load_ab(b, hh) for hh in range(2)}
        ebts, vps = {}, {}
        for h in range(H):
            c = h // 2
            hb = (h % 2) * DH
            if h + 2 < H:
                abL[h + 2] = load_ab(b, h + 2)
            sc = scp.tile([128, TOK], f32, tag="sc", name="sc")
            sc_ns = sc.rearrange("p (s n) -> p n s", s=S)
            # bias kron matmuls -> [128,512] psum tiles; ACT-copy into sc
            for g in range(4):
                psb = pp.tile([128, 512], f32, tag="pp", name="ps_b")
                for j in range(4):
                    nb = g * 4 + j
                    nc.tensor.matmul(
                        psb[:, j * 128:(j + 1) * 128],
                        lhsT=abL[h][:, nb * 128:(nb + 1) * 128],
                        rhs=wdh[:],
                        start=True, stop=True, skip_group_check=True)
                nc.scalar.copy(
                    sc_ns[:, g * 32:(g + 1) * 32, :],
                    psb.rearrange("p (n s) -> p n s", s=SP)[:, :, :S])
            # score matmuls accumulate onto the bias init
            for s in range(S):
                nc.tensor.matmul(
                    sc[:, s * 128:(s + 1) * 128],
                    lhsT=kh[c][hb:hb + DH, s::S],
                    rhs=qh[c][hb:hb + DH, s::S],
                    start=False, stop=True, skip_group_check=True)
            # exp -> bf16
            ebt = ebp.tile([128, TOK], bf16, tag="eb", name="ebt")
            nc.scalar.activation(ebt[:], sc[:], AF.Exp)
            # Z per (m, s); reciprocal; vp = vh_h * (1/Z) broadcast over d
            zt = zrp.tile([128, S], f32, tag="z", name="zt")
            nc.vector.reduce_sum(
                zt.unsqueeze(2),
                ebt.rearrange("p (s n) -> p s n", s=S),
                axis=mybir.AxisListType.X)
            rt = zrp.tile([128, S], f32, tag="r", name="rt")
            nc.vector.reciprocal(rt[:], zt[:])
            vp = vpp.tile([128, S * DH], bf16, tag="vp", name="vp")
            vsrc = vh.rearrange("p (s d) -> p s d", s=S)[
                :, :, h * DH:(h + 1) * DH]
            a2, b2 = broadcast_tensor_aps(vsrc, rt.unsqueeze(2))
            nc.gpsimd.tensor_mul(
                vp.rearrange("p (s d) -> p s d", s=S), a2, b2)
            ebts[h], vps[h] = ebt, vp
            # AV for the pair once both heads are done
            if h % 2 == 1:
                for g in range(4):
                    s0 = g * 4
                    sw = min(4, S - s0)
                    av = avp.tile([128, 512], f32, tag="av", name="av")
                    for hh in (h - 1, h):
                        hbb = (hh % 2) * DH
                        for si in range(sw):
                            s = s0 + si
                            nc.tensor.matmul(
                                av[hbb:hbb + DH, si * 128:(si + 1) * 128],
                                lhsT=vps[hh][:, s * DH:(s + 1) * DH],
                                rhs=ebts[hh][:, s * 128:(s + 1) * 128],
                                start=True, stop=True, skip_group_check=True)
                    nc.vector.tensor_copy(
                        yt[c][:, s0 * 128:(s0 + sw) * 128],
                        av[:, :sw * 128])

        # ---- P4: output projection
        for s in range(S):
            ps = pp.tile([128, D], f32, tag="pp", name="ps_o")
            for ci in range(C):
                nc.tensor.matmul(
                    ps[:],
                    lhsT=yt[ci][:, s * 128:(s + 1) * 128],
                    rhs=wo[ci][:],
                    start=(ci == 0), stop=(ci == C - 1))
            osb = osbp.tile([128, D], f32, tag="osb", name="osb")
            nc.vector.tensor_add(osb[:], ps[:], bob[:])
            nc.sync.dma_start(out_d[b, s], osb[:])


def build_nc():
    nc = bacc.Bacc("TRN2", target_bir_lowering=False, debug=False,
                   num_devices=NCORES)
    io = {}
    io["q"] = nc.dram_tensor("q", [B_LOC, D, N, S], bf16, kind="ExternalInput").ap()
    io["k"] = nc.dram_tensor("k", [B_LOC, D, N, S], bf16, kind="ExternalInput").ap()
    io["v"] = nc.dram_tensor("v", [B_LOC, D, N, S], bf16, kind="ExternalInput").ap()
    io["abk"] = nc.dram_tensor("abk", [B_LOC, H, K96, NBK * 128], bf16,
                               kind="ExternalInput").ap()
    for nm in ("wqT", "wkT", "wvT", "woT"):
        io[nm] = nc.dram_tensor(nm, [D, D], bf16, kind="ExternalInput").ap()
    io["cst"] = nc.dram_tensor("cst", [128, 8], f32, kind="ExternalInput").ap()
    for nm in ("bv", "bo"):
        io[nm] = nc.dram_tensor(nm, [D], f32, kind="ExternalInput").ap()
    io["wdh"] = nc.dram_tensor("wdh", [K96, NI * SP], bf16,
                               kind="ExternalInput").ap()
    io["out"] = nc.dram_tensor("out", [B_LOC, S, N, D], f32,
                               kind="ExternalOutput").ap()

    with tile.TileContext(nc) as tc:
        with ExitStack() as ctx:
            emit_kernel(ctx, tc, io)
    nc.compile()
    return nc


def host_prep(Wq, bq, Wk, bk, Wv, bv, Wd, bd, Wo, bo):
    """Pre-transpose weights (bf16); fold the qk scale into Wq; build the
    kron weight for the bias L->S projection; pack bq/bk per-partition.
    bd cancels in the softmax (constant along the normalized axis)."""
    bf = ml_dtypes.bfloat16
    scale = (D // H) ** -0.5
    prep = {
        "wqT": np.ascontiguousarray((np.asarray(Wq) * scale).T).astype(bf),
        "wkT": np.ascontiguousarray(np.asarray(Wk).T).astype(bf),
        "wvT": np.ascontiguousarray(np.asarray(Wv).T).astype(bf),
        "woT": np.ascontiguousarray(np.asarray(Wo).T).astype(bf),
        "bv": np.asarray(bv, np.float32),
        "bo": np.asarray(bo, np.float32),
    }
    cst = np.zeros((128, 8), np.float32)
    for c in range(C):
        cst[:, c] = np.asarray(bq[c * 128:(c + 1) * 128]) * scale
        cst[:, 4 + c] = np.asarray(bk[c * 128:(c + 1) * 128])
    prep["cst"] = cst
    # wdh[(l*NI+ni), (ni*SP+s)] = Wd[s, l]
    wdh = np.zeros((K96, NI * SP), np.float32)
    WdT = np.asarray(Wd, np.float32)  # [S, L]
    for l in range(L):
        for ni in range(NI):
            wdh[l * NI + ni, ni * SP:ni * SP + S] = WdT[:, l]
    prep["wdh"] = wdh.astype(bf)
    return prep


_NC_CACHE = None


def run(q, k, v, attn_bias, Wq, bq, Wk, bk, Wv, bv, Wd, bd, Wo, bo,
        trace=False, **trace_kwargs):
    global _NC_CACHE
    from concourse.bass_utils import run_bass_kernel_spmd

    if _NC_CACHE is None:
        _NC_CACHE = build_nc()
    nc = _NC_CACHE

    bf = ml_dtypes.bfloat16
    prep = host_prep(Wq, bq, Wk, bk, Wv, bv, Wd, bd, Wo, bo)
    q = np.asarray(q, np.float32).astype(bf)
    k = np.asarray(k, np.float32).astype(bf)
    v = np.asarray(v, np.float32).astype(bf)
    # ab [B, L, n, m, H] -> kron layout [B, H, (L, ni), (nb, m)]
    ab = np.asarray(attn_bias, np.float32)
    B = ab.shape[0]
    abk = ab.transpose(0, 4, 1, 2, 3)                 # [B, H, L, n, m]
    abk = abk.reshape(B, H, L, NBK, NI, N)            # n -> (nb, ni)
    abk = abk.transpose(0, 1, 2, 4, 3, 5)             # [B, H, L, ni, nb, m]
    abk = np.ascontiguousarray(abk).reshape(B, H, K96, NBK * 128).astype(bf)

    in_maps = []
    for i in range(NCORES):
        sl = slice(i * B_LOC, (i + 1) * B_LOC)
        in_maps.append({
            "q": np.ascontiguousarray(q[sl]),
            "k": np.ascontiguousarray(k[sl]),
            "v": np.ascontiguousarray(v[sl]),
            "abk": np.ascontiguousarray(abk[sl]),
            **prep,
        })
    res = run_bass_kernel_spmd(nc, in_maps, list(range(NCORES)), trace=trace,
                               **trace_kwargs)
    out = np.concatenate([res.results[i]["out"] for i in range(NCORES)], axis=0)
    return out, res


def kernel(**inputs):
    return run(**inputs)[0]


# revision 20
# speedup vs baseline: 5.2516x; 1.0072x over previous
"""Trainium2 Bass kernel for nn_MultiHeadAttention_86079734546451.

Sharding: data-parallel over batch B=16 across 8 cores (2 batches/core).
All weights replicated. No collectives.

Per-core math (B_loc=2, D=512, N=128 nodes, S=14, L=12, H=8, dh=64):
  qh/kh = d-major [dout, (n,s)] bf16 projections (scale folded into Wq).
  bias  = einsum('lnmh,sl->mns', ab, Wd) via per-(h,nb) kron matmuls:
     lhsT = host-pretransposed ab block [(l,ni)=96, m=128] bf16,
     rhs  = kron Wd [96, (ni,s16)];  4 blocks -> one [128,512] psum tile,
     ACT-copied (strided, dropping the s-pad) into the score psum as the
     accumulation INIT; score matmuls then accumulate with start=False.
     (bd cancels in the softmax and is dropped.)
  softmax over the query axis == free-axis softmax in the [m,(s,n)] layout:
     one exp [128,1792] -> ebt bf16, 3D reduce -> Z, reciprocal,
     gpsimd broadcast-multiply onto vh -> vp; AV matmuls per (h,s);
     O-projection from the d-major yt tiles.
"""

import sys

sys.path.insert(0, "/opt/trn_rl_repo")

from contextlib import ExitStack

import numpy as np
import ml_dtypes

import concourse.bass as bass
import concourse.mybir as mybir
import concourse.tile as tile
from concourse import bacc
from concourse.bass import broadcast_tensor_aps

f32 = mybir.dt.float32
bf16 = mybir.dt.bfloat16
AF = mybir.ActivationFunctionType

# Problem constants
B_LOC = 2          # batches per core
D = 512
N = 128            # nodes
S = 14             # seq
SP = 16            # padded S in the kron weight
L = 12
H = 8
DH = 64            # head dim
TOK = N * S        # 1792 tokens per batch, (n, s) order
C = 4              # 128-chunks of D
NCORES = 8
NI = 8             # n per kron block
NBK = N // NI      # 16 kron blocks
K96 = L * NI       # kron contraction size (real rows)
KB = 128           # padded kron K (zero rows 96..127) -> enables PE FWL


def emit_kernel(ctx: ExitStack, tc: "tile.TileContext", io: dict):
    nc = tc.nc

    q_d, k_d, v_d, ab_d = io["q"], io["k"], io["v"], io["abk"]
    out_d = io["out"]

    # ---------------- pools ----------------
    wpool = ctx.enter_context(tc.tile_pool(name="wpool", bufs=1))
    xin = ctx.enter_context(tc.tile_pool(name="xin", bufs=14))
    qkh = ctx.enter_context(tc.tile_pool(name="qkh", bufs=16))
    vhp = ctx.enter_context(tc.tile_pool(name="vhp", bufs=2))
    abp = ctx.enter_context(tc.tile_pool(name="abp", bufs=3))
    ebp = ctx.enter_context(tc.tile_pool(name="ebp", bufs=3))
    vpp = ctx.enter_context(tc.tile_pool(name="vpp", bufs=3))
    zrp = ctx.enter_context(tc.tile_pool(name="zrp", bufs=4))
    ytp = ctx.enter_context(tc.tile_pool(name="ytp", bufs=4))
    osbp = ctx.enter_context(tc.tile_pool(name="osbp", bufs=3))

    pp = ctx.enter_context(tc.tile_pool(name="pp", bufs=2, space="PSUM"))
    scp = ctx.enter_context(tc.tile_pool(name="scp", bufs=1, space="PSUM"))
    avp = ctx.enter_context(tc.tile_pool(name="avp", bufs=2, space="PSUM"))

    # ---------------- weights (once) ----------------
    wq, wk, wv, wo = [], [], [], []
    for c in range(C):
        for nm, lst in (("wqT", wq), ("wkT", wk), ("wvT", wv), ("woT", wo)):
            w_c = wpool.tile([128, D], bf16, name=f"{nm}{c}", tag=f"{nm}{c}")
            nc.scalar.dma_start(w_c[:], io[nm][c * 128:(c + 1) * 128, :])
            lst.append(w_c)

    wdh = wpool.tile([KB, NI * SP], bf16, name="wdh", tag="wdh")
    nc.scalar.dma_start(wdh[:], io["wdh"][:])

    # packed per-partition consts: cols 0-3 bq chunks, 4-7 bk chunks
    cst = wpool.tile([128, 8], f32, name="cst", tag="cst")
    nc.scalar.dma_start(cst[:], io["cst"][:])

    # broadcast bv/bo along partitions via K=1 ones-matmul
    ones = wpool.tile([1, 128], f32, name="ones", tag="ones")
    nc.vector.memset(ones[:], 1.0)
    bv_st = wpool.tile([1, D], f32, name="bv_st", tag="bv_st")
    nc.scalar.dma_start(bv_st[:], io["bv"].unsqueeze(0))
    bo_st = wpool.tile([1, D], f32, name="bo_st", tag="bo_st")
    nc.scalar.dma_start(bo_st[:], io["bo"].unsqueeze(0))
    bvb = wpool.tile([128, D], f32, name="bvb", tag="bvb")
    bob = wpool.tile([128, D], f32, name="bob", tag="bob")
    for src, dst in ((bv_st, bvb), (bo_st, bob)):
        pst = pp.tile([128, D], f32, tag="pp", name="p_bcast")
        nc.tensor.matmul(pst[:], lhsT=ones[:], rhs=src[:], start=True, stop=True)
        nc.vector.tensor_copy(dst[:], pst[:])

    def load_x(b):
        xs = {}
        for (src_d, nm) in ((q_d, "xq"), (k_d, "xk"), (v_d, "xv")):
            lst = []
            for ci in range(C):
                x_c = xin.tile([128, TOK], bf16, tag="xin", name=f"{nm}{ci}")
                nc.sync.dma_start(
                    x_c[:],
                    src_d[b, ci * 128:(ci + 1) * 128].rearrange(
                        "p n s -> p (n s)"))
                lst.append(x_c)
            xs[nm] = lst
        return xs

    def load_ab(b, h):
        ab_h = abp.tile([KB, NBK * 128], bf16, tag="abt", name=f"ab{h}")
        nc.sync.dma_start(ab_h[:], ab_d[b, h])
        return ab_h

    # ---------------- per-batch body ----------------
    xs = load_x(0)
    for b in range(B_LOC):
        xq, xk, xv = xs["xq"], xs["xk"], xs["xv"]

        # ---- P1: Q and K projections -> d-major [dout, (n,s)] bf16
        qh, kh = [], []
        for pi, (xin_l, wts, dst_list) in enumerate(
                ((xq, wq, qh), (xk, wk, kh))):
            for co in range(C):
                h_c = qkh.tile([128, TOK], bf16, tag="qkh", name=f"h{co}")
                for tb in range(4):
                    ps = pp.tile([128, 448], f32, tag="pp", name="ps_qk")
                    for ci in range(C):
                        nc.tensor.matmul(
                            ps[:],
                            lhsT=wts[ci][:, co * 128:(co + 1) * 128],
                            rhs=xin_l[ci][:, tb * 448:(tb + 1) * 448],
                            start=(ci == 0), stop=(ci == C - 1))
                    eng = nc.scalar if pi == 0 else nc.vector
                    if pi == 0:
                        nc.scalar.activation(
                            h_c[:, tb * 448:(tb + 1) * 448], ps[:],
                            AF.Identity, bias=cst[:, co:co + 1], scale=1.0)
                    else:
                        nc.vector.tensor_scalar_add(
                            h_c[:, tb * 448:(tb + 1) * 448], ps[:],
                            cst[:, 4 + co:4 + co + 1])
                dst_list.append(h_c)

        # ---- P1b: V projection -> token-major vh [n, (s,d)] bf16 (+bv)
        vh = vhp.tile([128, S * D], bf16, tag="vh", name="vh")
        for s in range(S):
            ps = pp.tile([128, D], f32, tag="pp", name="ps_v")
            for ci in range(C):
                nc.tensor.matmul(
                    ps[:],
                    lhsT=xv[ci][:, s::S],
                    rhs=wv[ci][:],
                    start=(ci == 0), stop=(ci == C - 1))
            nc.vector.tensor_add(vh[:, s * D:(s + 1) * D], ps[:], bvb[:])

        # prefetch next batch's inputs (sync queue, before this batch's outs)
        if b + 1 < B_LOC:
            xs = load_x(b + 1)

        # ---- P3: attention per head
        yt = [ytp.tile([128, TOK], bf16, tag="ytp", name=f"yt{c}")
              for c in range(C)]
        abL = {hh: load_ab(b, hh) for hh in range(2)}
        ebts, vps = {}, {}

        def av_block(pair):
            cc = pair
            for g in range(4):
                s0 = g * 4
                sw = min(4, S - s0)
                av = avp.tile([128, 512], f32, tag="av", name="av")
                for hh in (2 * cc, 2 * cc + 1):
                    hbb = (hh % 2) * DH
                    for si in range(sw):
                        s = s0 + si
                        nc.tensor.matmul(
                            av[hbb:hbb + DH, si * 128:(si + 1) * 128],
                            lhsT=vps[hh][:, s * DH:(s + 1) * DH],
                            rhs=ebts[hh][:, s * 128:(s + 1) * 128],
                            start=True, stop=True, skip_group_check=True)
                nc.vector.tensor_copy(
                    yt[cc][:, s0 * 128:(s0 + sw) * 128],
                    av[:, :sw * 128])

        for h in range(H):
            c = h // 2
            hb = (h % 2) * DH
            if h + 2 < H:
                abL[h + 2] = load_ab(b, h + 2)
            sc = scp.tile([128, TOK], f32, tag="sc", name="sc")
            sc_ns = sc.rearrange("p (s n) -> p n s", s=S)
            # bias kron matmuls -> [128,512] psum tiles; ACT-copy into sc
            for g in range(4):
                psb = pp.tile([128, 512], f32, tag="pp", name="ps_b")
                for j in range(4):
                    nb = g * 4 + j
                    nc.tensor.matmul(
                        psb[:, j * 128:(j + 1) * 128],
                        lhsT=abL[h][:, nb * 128:(nb + 1) * 128],
                        rhs=wdh[:],
                        start=True, stop=True, skip_group_check=True)
                nc.scalar.copy(
                    sc_ns[:, g * 32:(g + 1) * 32, :],
                    psb.rearrange("p (n s) -> p n s", s=SP)[:, :, :S])
            # AV of the previous pair slots in here: its vp/ebt chain has
            # had a full head iteration to finish, so PE never stalls on it
            if h % 2 == 0 and h >= 2:
                av_block(h // 2 - 1)
            # score matmuls accumulate onto the bias init
            for s in range(S):
                nc.tensor.matmul(
                    sc[:, s * 128:(s + 1) * 128],
                    lhsT=kh[c][hb:hb + DH, s::S],
                    rhs=qh[c][hb:hb + DH, s::S],
                    start=False, stop=True, skip_group_check=True)
            # exp -> bf16
            ebt = ebp.tile([128, TOK], bf16, tag="eb", name="ebt")
            nc.scalar.activation(ebt[:], sc[:], AF.Exp)
            # Z per (m, s); reciprocal; vp = vh_h * (1/Z) broadcast over d
            zt = zrp.tile([128, S], f32, tag="z", name="zt")
            nc.vector.reduce_sum(
                zt.unsqueeze(2),
                ebt.rearrange("p (s n) -> p s n", s=S),
                axis=mybir.AxisListType.X)
            rt = zrp.tile([128, S], f32, tag="r", name="rt")
            nc.vector.reciprocal(rt[:], zt[:])
            vp = vpp.tile([128, S * DH], bf16, tag="vp", name="vp")
            vsrc = vh.rearrange("p (s d) -> p s d", s=S)[
                :, :, h * DH:(h + 1) * DH]
            a2, b2 = broadcast_tensor_aps(vsrc, rt.unsqueeze(2))
            nc.gpsimd.tensor_mul(
                vp.rearrange("p (s d) -> p s d", s=S), a2, b2)
            ebts[h], vps[h] = ebt, vp
        av_block(3)

        # ---- P4: output projection
        for s in range(S):
            ps = pp.tile([128, D], f32, tag="pp", name="ps_o")
            for ci in range(C):
                nc.tensor.matmul(
                    ps[:],
                    lhsT=yt[ci][:, s * 128:(s + 1) * 128],
                    rhs=wo[ci][:],
                    start=(ci == 0), stop=(ci == C - 1))
            osb = osbp.tile([128, D], f32, tag="osb", name="osb")
            nc.vector.tensor_add(osb[:], ps[:], bob[:])
            nc.sync.dma_start(out_d[b, s], osb[:])


def build_nc():
    nc = bacc.Bacc("TRN2", target_bir_lowering=False, debug=False,
                   num_devices=NCORES)
    io = {}
    io["q"] = nc.dram_tensor("q", [B_LOC, D, N, S], bf16, kind="ExternalInput").ap()
    io["k"] = nc.dram_tensor("k", [B_LOC, D, N, S], bf16, kind="ExternalInput").ap()
    io["v"] = nc.dram_tensor("v", [B_LOC, D, N, S], bf16, kind="ExternalInput").ap()
    io["abk"] = nc.dram_tensor("abk", [B_LOC, H, KB, NBK * 128], bf16,
                               kind="ExternalInput").ap()
    for nm in ("wqT", "wkT", "wvT", "woT"):
        io[nm] = nc.dram_tensor(nm, [D, D], bf16, kind="ExternalInput").ap()
    io["cst"] = nc.dram_tensor("cst", [128, 8], f32, kind="ExternalInput").ap()
    for nm in ("bv", "bo"):
        io[nm] = nc.dram_tensor(nm, [D], f32, kind="ExternalInput").ap()
    io["wdh"] = nc.dram_tensor("wdh", [KB, NI * SP], bf16,
                               kind="ExternalInput").ap()
    io["out"] = nc.dram_tensor("out", [B_LOC, S, N, D], f32,
                               kind="ExternalOutput").ap()

    with tile.TileContext(nc) as tc:
        with ExitStack() as ctx:
            emit_kernel(ctx, tc, io)
    nc.compile()
    return nc


def host_prep(Wq, bq, Wk, bk, Wv, bv, Wd, bd, Wo, bo):
    """Pre-transpose weights (bf16); fold the qk scale into Wq; build the
    kron weight for the bias L->S projection; pack bq/bk per-partition.
    bd cancels in the softmax (constant along the normalized axis)."""
    bf = ml_dtypes.bfloat16
    scale = (D // H) ** -0.5
    prep = {
        "wqT": np.ascontiguousarray((np.asarray(Wq) * scale).T).astype(bf),
        "wkT": np.ascontiguousarray(np.asarray(Wk).T).astype(bf),
        "wvT": np.ascontiguousarray(np.asarray(Wv).T).astype(bf),
        "woT": np.ascontiguousarray(np.asarray(Wo).T).astype(bf),
        "bv": np.asarray(bv, np.float32),
        "bo": np.asarray(bo, np.float32),
    }
    cst = np.zeros((128, 8), np.float32)
    for c in range(C):
        cst[:, c] = np.asarray(bq[c * 128:(c + 1) * 128]) * scale
        cst[:, 4 + c] = np.asarray(bk[c * 128:(c + 1) * 128])
    prep["cst"] = cst
    # wdh[(l*NI+ni), (ni*SP+s)] = Wd[s, l]; rows 96..127 zero pad (FWL)
    wdh = np.zeros((KB, NI * SP), np.float32)
    WdT = np.asarray(Wd, np.float32)  # [S, L]
    for l in range(L):
        for ni in range(NI):
            wdh[l * NI + ni, ni * SP:ni * SP + S] = WdT[:, l]
    prep["wdh"] = wdh.astype(bf)
    return prep


_NC_CACHE = None


def run(q, k, v, attn_bias, Wq, bq, Wk, bk, Wv, bv, Wd, bd, Wo, bo,
        trace=False, **trace_kwargs):
    global _NC_CACHE
    from concourse.bass_utils import run_bass_kernel_spmd

    if _NC_CACHE is None:
        _NC_CACHE = build_nc()
    nc = _NC_CACHE

    bf = ml_dtypes.bfloat16
    prep = host_prep(Wq, bq, Wk, bk, Wv, bv, Wd, bd, Wo, bo)
    q = np.asarray(q, np.float32).astype(bf)
    k = np.asarray(k, np.float32).astype(bf)
    v = np.asarray(v, np.float32).astype(bf)
    # ab [B, L, n, m, H] -> kron layout [B, H, (L, ni), (nb, m)]
    ab = np.asarray(attn_bias, np.float32)
    B = ab.shape[0]
    abk = ab.transpose(0, 4, 1, 2, 3)                 # [B, H, L, n, m]
    abk = abk.reshape(B, H, L, NBK, NI, N)            # n -> (nb, ni)
    abk = abk.transpose(0, 1, 2, 4, 3, 5)             # [B, H, L, ni, nb, m]
    abk = np.ascontiguousarray(abk).reshape(B, H, K96, NBK * 128).astype(bf)
    abk = np.concatenate(
        [abk, np.zeros((B, H, KB - K96, NBK * 128), bf)], axis=2)

    in_maps = []
    for i in range(NCORES):
        sl = slice(i * B_LOC, (i + 1) * B_LOC)
        in_maps.append({
            "q": np.ascontiguousarray(q[sl]),
            "k": np.ascontiguousarray(k[sl]),
            "v": np.ascontiguousarray(v[sl]),
            "abk": np.ascontiguousarray(abk[sl]),
            **prep,
        })
    res = run_bass_kernel_spmd(nc, in_maps, list(range(NCORES)), trace=trace,
                               **trace_kwargs)
    out = np.concatenate([res.results[i]["out"] for i in range(NCORES)], axis=0)
    return out, res


def kernel(**inputs):
    return run(**inputs)[0]


# revision 25
# speedup vs baseline: 5.7533x; 1.0955x over previous
"""Trainium2 Bass kernel for nn_MultiHeadAttention_86079734546451.

Sharding: data-parallel over batch B=16 across 8 cores (2 batches/core).
All weights replicated. No collectives.

Per-core math (B_loc=2, D=512, N=128 nodes, S=14, L=12, H=8, dh=64):
  qh/kh = d-major [dout, (n,s)] bf16 projections (scale folded into Wq).
  bias  = einsum('lnmh,sl->mns', ab, Wd) via per-(h,nb) kron matmuls:
     lhsT = host-pretransposed ab block [(l,ni)=96, m=128] bf16,
     rhs  = kron Wd [96, (ni,s16)];  4 blocks -> one [128,512] psum tile,
     ACT-copied (strided, dropping the s-pad) into the score psum as the
     accumulation INIT; score matmuls then accumulate with start=False.
     (bd cancels in the softmax and is dropped.)
  softmax over the query axis == free-axis softmax in the [m,(s,n)] layout:
     one exp [128,1792] -> ebt bf16, 3D reduce -> Z, reciprocal,
     gpsimd broadcast-multiply onto vh -> vp; AV matmuls per (h,s);
     O-projection from the d-major yt tiles.
"""

import sys

sys.path.insert(0, "/opt/trn_rl_repo")

from contextlib import ExitStack

import numpy as np
import ml_dtypes

import concourse.bass as bass
import concourse.mybir as mybir
import concourse.tile as tile
from concourse import bacc
from concourse.bass import broadcast_tensor_aps

f32 = mybir.dt.float32
bf16 = mybir.dt.bfloat16
AF = mybir.ActivationFunctionType

# Problem constants
B_LOC = 2          # batches per core
D = 512
N = 128            # nodes
S = 14             # seq
SP = 16            # padded S in the kron weight
L = 12
H = 8
DH = 64            # head dim
TOK = N * S        # 1792 tokens per batch, (n, s) order
C = 4              # 128-chunks of D
NCORES = 8
NI = 8             # n per kron block
NBK = N // NI      # 16 kron blocks
K96 = L * NI       # kron contraction size (real rows)
KB = 128           # padded kron K (zero rows 96..127) -> enables PE FWL


def emit_kernel(ctx: ExitStack, tc: "tile.TileContext", io: dict):
    nc = tc.nc

    q_d, k_d, v_d, ab_d = io["q"], io["k"], io["v"], io["abk"]
    out_d = io["out"]

    # ---------------- pools ----------------
    wpool = ctx.enter_context(tc.tile_pool(name="wpool", bufs=1))
    xin = ctx.enter_context(tc.tile_pool(name="xin", bufs=14))
    qkh = ctx.enter_context(tc.tile_pool(name="qkh", bufs=16))
    vhp = ctx.enter_context(tc.tile_pool(name="vhp", bufs=2))
    abp = ctx.enter_context(tc.tile_pool(name="abp", bufs=3))
    ebp = ctx.enter_context(tc.tile_pool(name="ebp", bufs=3))
    vpp = ctx.enter_context(tc.tile_pool(name="vpp", bufs=3))
    zrp = ctx.enter_context(tc.tile_pool(name="zrp", bufs=4))
    ytp = ctx.enter_context(tc.tile_pool(name="ytp", bufs=4))
    osbp = ctx.enter_context(tc.tile_pool(name="osbp", bufs=3))

    pp = ctx.enter_context(tc.tile_pool(name="pp", bufs=2, space="PSUM"))
    scp = ctx.enter_context(tc.tile_pool(name="scp", bufs=1, space="PSUM"))
    avp = ctx.enter_context(tc.tile_pool(name="avp", bufs=2, space="PSUM"))

    # ---------------- weights (once) ----------------
    wq, wk, wv, wo = [], [], [], []
    for c in range(C):
        for nm, lst in (("wqT", wq), ("wkT", wk), ("wvT", wv), ("woT", wo)):
            w_c = wpool.tile([128, D], bf16, name=f"{nm}{c}", tag=f"{nm}{c}")
            nc.scalar.dma_start(w_c[:], io[nm][c * 128:(c + 1) * 128, :])
            lst.append(w_c)

    wdh = wpool.tile([KB, NI * SP], bf16, name="wdh", tag="wdh")
    nc.scalar.dma_start(wdh[:], io["wdh"][:])

    # packed per-partition consts: cols 0-3 bq chunks, 4-7 bk chunks
    cst = wpool.tile([128, 8], f32, name="cst", tag="cst")
    nc.scalar.dma_start(cst[:], io["cst"][:])

    # broadcast bv/bo along partitions via K=1 ones-matmul
    ones = wpool.tile([1, 128], f32, name="ones", tag="ones")
    nc.vector.memset(ones[:], 1.0)
    bv_st = wpool.tile([1, D], f32, name="bv_st", tag="bv_st")
    nc.scalar.dma_start(bv_st[:], io["bv"].unsqueeze(0))
    bo_st = wpool.tile([1, D], f32, name="bo_st", tag="bo_st")
    nc.scalar.dma_start(bo_st[:], io["bo"].unsqueeze(0))
    bvb = wpool.tile([128, D], f32, name="bvb", tag="bvb")
    bob = wpool.tile([128, D], f32, name="bob", tag="bob")
    for src, dst in ((bv_st, bvb), (bo_st, bob)):
        pst = pp.tile([128, D], f32, tag="pp", name="p_bcast")
        nc.tensor.matmul(pst[:], lhsT=ones[:], rhs=src[:], start=True, stop=True)
        nc.vector.tensor_copy(dst[:], pst[:])

    def load_x(b):
        # x tiles are [128, (s, n)] -- host pre-transposed to [B, D, S, N]
        xs = {}
        for (src_d, nm) in ((q_d, "xq"), (k_d, "xk"), (v_d, "xv")):
            lst = []
            for ci in range(C):
                x_c = xin.tile([128, TOK], bf16, tag="xin", name=f"{nm}{ci}")
                nc.sync.dma_start(
                    x_c[:],
                    src_d[b, ci * 128:(ci + 1) * 128].rearrange(
                        "p s n -> p (s n)"))
                lst.append(x_c)
            xs[nm] = lst
        return xs

    def load_ab(b, h):
        ab_h = abp.tile([KB, NBK * 128], bf16, tag="abt", name=f"ab{h}")
        nc.sync.dma_start(ab_h[:], ab_d[b, h])
        return ab_h

    # ---------------- per-batch body ----------------
    xs = load_x(0)
    for b in range(B_LOC):
        xq, xk, xv = xs["xq"], xs["xk"], xs["xv"]

        # ---- P1: Q and K projections -> d-major [dout, (n,s)] bf16
        qh, kh = [], []
        for pi, (xin_l, wts, dst_list) in enumerate(
                ((xq, wq, qh), (xk, wk, kh))):
            for co in range(C):
                h_c = qkh.tile([128, TOK], bf16, tag="qkh", name=f"h{co}")
                for tb in range(4):
                    ps = pp.tile([128, 448], f32, tag="pp", name="ps_qk")
                    for ci in range(C):
                        nc.tensor.matmul(
                            ps[:],
                            lhsT=wts[ci][:, co * 128:(co + 1) * 128],
                            rhs=xin_l[ci][:, tb * 448:(tb + 1) * 448],
                            start=(ci == 0), stop=(ci == C - 1))
                    eng = nc.scalar if pi == 0 else nc.vector
                    if pi == 0:
                        nc.scalar.activation(
                            h_c[:, tb * 448:(tb + 1) * 448], ps[:],
                            AF.Identity, bias=cst[:, co:co + 1], scale=1.0)
                    else:
                        nc.vector.tensor_scalar_add(
                            h_c[:, tb * 448:(tb + 1) * 448], ps[:],
                            cst[:, 4 + co:4 + co + 1])
                dst_list.append(h_c)

        # ---- P1b: V projection -> token-major vh [n, (s,d)] bf16 (+bv)
        vh = vhp.tile([128, S * D], bf16, tag="vh", name="vh")
        for s in range(S):
            ps = pp.tile([128, D], f32, tag="pp", name="ps_v")
            for ci in range(C):
                nc.tensor.matmul(
                    ps[:],
                    lhsT=xv[ci][:, s * 128:(s + 1) * 128],
                    rhs=wv[ci][:],
                    start=(ci == 0), stop=(ci == C - 1))
            nc.vector.tensor_add(vh[:, s * D:(s + 1) * D], ps[:], bvb[:])

        # prefetch next batch's inputs (sync queue, before this batch's outs)
        if b + 1 < B_LOC:
            xs = load_x(b + 1)

        # ---- P3: attention per head
        yt = [ytp.tile([128, TOK], bf16, tag="ytp", name=f"yt{c}")
              for c in range(C)]
        abL = {hh: load_ab(b, hh) for hh in range(2)}
        ebts, vps = {}, {}

        def av_block(pair):
            cc = pair
            for g in range(4):
                s0 = g * 4
                sw = min(4, S - s0)
                av = avp.tile([128, 512], f32, tag="av", name="av")
                for hh in (2 * cc, 2 * cc + 1):
                    hbb = (hh % 2) * DH
                    for si in range(sw):
                        s = s0 + si
                        nc.tensor.matmul(
                            av[hbb:hbb + DH, si * 128:(si + 1) * 128],
                            lhsT=vps[hh][:, s * DH:(s + 1) * DH],
                            rhs=ebts[hh][:, s * 128:(s + 1) * 128],
                            start=True, stop=True, skip_group_check=True)
                nc.vector.tensor_copy(
                    yt[cc][:, s0 * 128:(s0 + sw) * 128],
                    av[:, :sw * 128])

        for h in range(H):
            c = h // 2
            hb = (h % 2) * DH
            if h + 2 < H:
                abL[h + 2] = load_ab(b, h + 2)
            sc = scp.tile([128, TOK], f32, tag="sc", name="sc")
            sc_ns = sc.rearrange("p (s n) -> p n s", s=S)
            # bias kron matmuls -> [128,512] psum tiles; ACT-copy into sc
            for g in range(4):
                psb = pp.tile([128, 512], f32, tag="pp", name="ps_b")
                for j in range(4):
                    nb = g * 4 + j
                    nc.tensor.matmul(
                        psb[:, j * 128:(j + 1) * 128],
                        lhsT=abL[h][:, nb * 128:(nb + 1) * 128],
                        rhs=wdh[:],
                        start=True, stop=True, skip_group_check=True)
                nc.scalar.copy(
                    sc_ns[:, g * 32:(g + 1) * 32, :],
                    psb.rearrange("p (n s) -> p n s", s=SP)[:, :, :S])
            # AV of the previous pair slots in here: its vp/ebt chain has
            # had a full head iteration to finish, so PE never stalls on it
            if h % 2 == 0 and h >= 2:
                av_block(h // 2 - 1)
            # score matmuls accumulate onto the bias init
            for s in range(S):
                nc.tensor.matmul(
                    sc[:, s * 128:(s + 1) * 128],
                    lhsT=kh[c][hb:hb + DH, s * 128:(s + 1) * 128],
                    rhs=qh[c][hb:hb + DH, s * 128:(s + 1) * 128],
                    start=False, stop=True, skip_group_check=True)
            # exp -> bf16
            ebt = ebp.tile([128, TOK], bf16, tag="eb", name="ebt")
            nc.scalar.activation(ebt[:], sc[:], AF.Exp)
            # Z per (m, s); reciprocal; vp = vh_h * (1/Z) broadcast over d
            zt = zrp.tile([128, S], f32, tag="z", name="zt")
            nc.vector.reduce_sum(
                zt.unsqueeze(2),
                ebt.rearrange("p (s n) -> p s n", s=S),
                axis=mybir.AxisListType.X)
            rt = zrp.tile([128, S], f32, tag="r", name="rt")
            nc.vector.reciprocal(rt[:], zt[:])
            vp = vpp.tile([128, S * DH], bf16, tag="vp", name="vp")
            vsrc = vh.rearrange("p (s d) -> p s d", s=S)[
                :, :, h * DH:(h + 1) * DH]
            a2, b2 = broadcast_tensor_aps(vsrc, rt.unsqueeze(2))
            nc.gpsimd.tensor_mul(
                vp.rearrange("p (s d) -> p s d", s=S), a2, b2)
            ebts[h], vps[h] = ebt, vp
        av_block(3)

        # ---- P4: output projection
        for s in range(S):
            ps = pp.tile([128, D], f32, tag="pp", name="ps_o")
            for ci in range(C):
                nc.tensor.matmul(
                    ps[:],
                    lhsT=yt[ci][:, s * 128:(s + 1) * 128],
                    rhs=wo[ci][:],
                    start=(ci == 0), stop=(ci == C - 1))
            osb = osbp.tile([128, D], f32, tag="osb", name="osb")
            nc.vector.tensor_add(osb[:], ps[:], bob[:])
            nc.sync.dma_start(out_d[b, s], osb[:])


def build_nc():
    nc = bacc.Bacc("TRN2", target_bir_lowering=False, debug=False,
                   num_devices=NCORES)
    io = {}
    io["q"] = nc.dram_tensor("q", [B_LOC, D, S, N], bf16, kind="ExternalInput").ap()
    io["k"] = nc.dram_tensor("k", [B_LOC, D, S, N], bf16, kind="ExternalInput").ap()
    io["v"] = nc.dram_tensor("v", [B_LOC, D, S, N], bf16, kind="ExternalInput").ap()
    io["abk"] = nc.dram_tensor("abk", [B_LOC, H, KB, NBK * 128], bf16,
                               kind="ExternalInput").ap()
    for nm in ("wqT", "wkT", "wvT", "woT"):
        io[nm] = nc.dram_tensor(nm, [D, D], bf16, kind="ExternalInput").ap()
    io["cst"] = nc.dram_tensor("cst", [128, 8], f32, kind="ExternalInput").ap()
    for nm in ("bv", "bo"):
        io[nm] = nc.dram_tensor(nm, [D], f32, kind="ExternalInput").ap()
    io["wdh"] = nc.dram_tensor("wdh", [KB, NI * SP], bf16,
                               kind="ExternalInput").ap()
    io["out"] = nc.dram_tensor("out", [B_LOC, S, N, D], f32,
                               kind="ExternalOutput").ap()

    with tile.TileContext(nc) as tc:
        with ExitStack() as ctx:
            emit_kernel(ctx, tc, io)
    nc.compile()
    return nc


def host_prep(Wq, bq, Wk, bk, Wv, bv, Wd, bd, Wo, bo):
    """Pre-transpose weights (bf16); fold the qk scale into Wq; build the
    kron weight for the bias L->S projection; pack bq/bk per-partition.
    bd cancels in the softmax (constant along the normalized axis)."""
    bf = ml_dtypes.bfloat16
    scale = (D // H) ** -0.5
    prep = {
        "wqT": np.ascontiguousarray((np.asarray(Wq) * scale).T).astype(bf),
        "wkT": np.ascontiguousarray(np.asarray(Wk).T).astype(bf),
        "wvT": np.ascontiguousarray(np.asarray(Wv).T).astype(bf),
        "woT": np.ascontiguousarray(np.asarray(Wo).T).astype(bf),
        "bv": np.asarray(bv, np.float32),
        "bo": np.asarray(bo, np.float32),
    }
    cst = np.zeros((128, 8), np.float32)
    for c in range(C):
        cst[:, c] = np.asarray(bq[c * 128:(c + 1) * 128]) * scale
        cst[:, 4 + c] = np.asarray(bk[c * 128:(c + 1) * 128])
    prep["cst"] = cst
    # wdh[(l*NI+ni), (ni*SP+s)] = Wd[s, l]; rows 96..127 zero pad (FWL)
    wdh = np.zeros((KB, NI * SP), np.float32)
    WdT = np.asarray(Wd, np.float32)  # [S, L]
    for l in range(L):
        for ni in range(NI):
            wdh[l * NI + ni, ni * SP:ni * SP + S] = WdT[:, l]
    prep["wdh"] = wdh.astype(bf)
    return prep


_NC_CACHE = None


def run(q, k, v, attn_bias, Wq, bq, Wk, bk, Wv, bv, Wd, bd, Wo, bo,
        trace=False, **trace_kwargs):
    global _NC_CACHE
    from concourse.bass_utils import run_bass_kernel_spmd

    if _NC_CACHE is None:
        _NC_CACHE = build_nc()
    nc = _NC_CACHE

    bf = ml_dtypes.bfloat16
    prep = host_prep(Wq, bq, Wk, bk, Wv, bv, Wd, bd, Wo, bo)
    # [B, D, N, S] -> [B, D, S, N] so on-chip token order is (s, n)
    q = np.ascontiguousarray(np.asarray(q, np.float32).transpose(0, 1, 3, 2)).astype(bf)
    k = np.ascontiguousarray(np.asarray(k, np.float32).transpose(0, 1, 3, 2)).astype(bf)
    v = np.ascontiguousarray(np.asarray(v, np.float32).transpose(0, 1, 3, 2)).astype(bf)
    # ab [B, L, n, m, H] -> kron layout [B, H, (L, ni), (nb, m)]
    ab = np.asarray(attn_bias, np.float32)
    B = ab.shape[0]
    abk = ab.transpose(0, 4, 1, 2, 3)                 # [B, H, L, n, m]
    abk = abk.reshape(B, H, L, NBK, NI, N)            # n -> (nb, ni)
    abk = abk.transpose(0, 1, 2, 4, 3, 5)             # [B, H, L, ni, nb, m]
    abk = np.ascontiguousarray(abk).reshape(B, H, K96, NBK * 128).astype(bf)
    abk = np.concatenate(
        [abk, np.zeros((B, H, KB - K96, NBK * 128), bf)], axis=2)

    in_maps = []
    for i in range(NCORES):
        sl = slice(i * B_LOC, (i + 1) * B_LOC)
        in_maps.append({
            "q": np.ascontiguousarray(q[sl]),
            "k": np.ascontiguousarray(k[sl]),
            "v": np.ascontiguousarray(v[sl]),
            "abk": np.ascontiguousarray(abk[sl]),
            **prep,
        })
    res = run_bass_kernel_spmd(nc, in_maps, list(range(NCORES)), trace=trace,
                               **trace_kwargs)
    out = np.concatenate([res.results[i]["out"] for i in range(NCORES)], axis=0)
    return out, res


def kernel(**inputs):
    return run(**inputs)[0]
